# revision 47
# baseline (speedup 1.0000x reference)
"""Windowed sparse point-transformer layer on 8 Trainium2 NeuronCores.

Strategy (spec sharding_hint): windows are independent, so the host
scatters voxels into dense per-window buffers (the "all-to-all keyed by
window id" done as a host-side relayout), shards 900->928 windows across
8 cores (116 each), and each core runs a dense windowed attention+FFN
Bass kernel. Weights are replicated. The host gathers per-window outputs
back to the sparse voxel list.

Device kernel per window (T=128 slots, C=48 channels, 8 heads x 6 dim):
  - q/k projections produce CHANNEL-major qT/kT with heads padded onto
    32-partition strips so the 8 score matmuls use PE row tiling
    (K=8 per head: 6 data lanes + a ones/mask lane that applies the
    -1e5 padding mask during the matmul itself; bias is folded in via
    the input's ones row).
  - softmax (q-major): exp on ACT with fused row-sum accumulation,
    reciprocal + per-head normalize on DVE (bf16), attn transposed on
    PE, ctx matmuls against bf16 v accumulate fp32 in PSUM.
  - residual 1: x.T is PE-transposed straight into the same PSUM
    accumulation group as the output projection (ctx @ Wo + bo).
  - FFN batched across a 4-window chunk (N=512) in float32r so the PE
    runs at full rate; b1/b2 ride the activation bias port during PSUM
    eviction. LayerNorms run token-major on DVE/ACT (bn_stats/bn_aggr).

PSUM is 8 banks; tile tags are assigned so the live set maps onto 8
bank-sized slots (s0-s3 shared by projections/scores/FFN, at, misc,
cx, y1).
"""

import numpy as np
import ml_dtypes
from contextlib import ExitStack

GX, GY, GZ = 120, 120, 8
WX, WY, WZ = 8, 8, 2
T = 128
C = 48
H = 8
HD = 6
FF = 256
N = 80000
NW = (GX // WX) * (GY // WY) * (GZ // WZ)  # 900
NCORES = 8
WPC = 116  # windows per core (900 padded to 928 = 8*116)
CHUNK = 4  # windows per projection/FFN batch (N=512 tokens)
MASKVAL = -1e5

_CACHE = {}


def _build_bass(wpc, reps=1, ablate=()):
    import concourse.bass as bass
    import concourse.tile as tile
    import concourse.mybir as mybir
    from concourse import bacc
    from concourse.masks import make_identity

    f32 = mybir.dt.float32
    f32r = mybir.dt.float32r
    bf16 = mybir.dt.bfloat16
    AF = mybir.ActivationFunctionType
    ALU = mybir.AluOpType

    nc = bacc.Bacc("TRN2", target_bir_lowering=False, debug=False)
    ntok = wpc * T

    xta = nc.dram_tensor("xta", [50, ntok], f32r, kind="ExternalInput")
    wqa = nc.dram_tensor("wqa", [50, 128], f32r, kind="ExternalInput")
    wqb = nc.dram_tensor("wqb", [50, 128], f32r, kind="ExternalInput")
    wka = nc.dram_tensor("wka", [50, 128], f32r, kind="ExternalInput")
    wkb = nc.dram_tensor("wkb", [50, 128], f32r, kind="ExternalInput")
    wv = nc.dram_tensor("wv", [50, 56], f32r, kind="ExternalInput")
    esel = nc.dram_tensor("esel", [128, 128], bf16, kind="ExternalInput")
    wopa = nc.dram_tensor("wopa", [128, 48], bf16, kind="ExternalInput")
    wopb = nc.dram_tensor("wopb", [128, 48], bf16, kind="ExternalInput")
    boc = nc.dram_tensor("boc", [48, 1], f32, kind="ExternalInput")
    w1 = nc.dram_tensor("w1", [48, 256], f32r, kind="ExternalInput")
    w2 = nc.dram_tensor("w2", [256, 48], f32r, kind="ExternalInput")
    b1c = nc.dram_tensor("b1c", [128, 2], f32, kind="ExternalInput")
    b2c = nc.dram_tensor("b2c", [48, 1], f32, kind="ExternalInput")
    lnc = nc.dram_tensor("lnc", [4, 48], f32, kind="ExternalInput")
    out = nc.dram_tensor("out", [wpc, T, C], f32, kind="ExternalOutput")

    nchunk = wpc // CHUNK
    assert wpc % CHUNK == 0

    def _bn(col, n):
        return bass.AP(tensor=col.tensor, offset=col.offset,
                       ap=[col.ap[0], [0, n]])

    def _b(ap3, n=None):
        # broadcast a [128, k] AP along a new trailing free dim of size 48
        return bass.AP(tensor=ap3.tensor, offset=ap3.offset,
                       ap=list(ap3.ap) + [[0, 48]])

    def ln_apply(pool, y_ap, g_ap, b_ap, eps_s, out_dtype, tagp):
        """LayerNorm over the free dim (C=48) of token-major y_ap."""
        mv = pool.tile([128, 2], f32, tag=f"{tagp}mv", name=f"{tagp}mv")
        st6 = pool.tile([128, 6], f32, tag=f"{tagp}st", name=f"{tagp}st")
        nc.vector.bn_stats(out=st6[:], in_=y_ap)
        nc.vector.bn_aggr(out=mv[:], in_=st6[:])
        rstd = pool.tile([128, 1], f32, tag=f"{tagp}rs", name=f"{tagp}rs")
        nc.scalar.activation(
            out=rstd[:], in_=mv[:, 1:2], func=AF.Ln, bias=eps_s[:]
        )
        nc.scalar.activation(
            out=rstd[:], in_=rstd[:], func=AF.Exp, scale=-0.5
        )
        yc = pool.tile([128, 48], f32, tag=f"{tagp}yc", name=f"{tagp}yc")
        nc.vector.tensor_scalar_sub(yc[:], y_ap, mv[:, 0:1])
        ho = pool.tile([128, 48], out_dtype, tag=f"{tagp}ho", name=f"{tagp}ho")
        nc.vector.scalar_tensor_tensor(
            out=ho[:], in0=yc[:], scalar=rstd[:], in1=g_ap,
            op0=ALU.mult, op1=ALU.mult,
        )
        nc.vector.tensor_add(ho[:], ho[:], b_ap)
        return ho

    with tile.TileContext(nc) as tc, ExitStack() as ctx:
        singles = ctx.enter_context(tc.tile_pool(name="singles", bufs=1))

        xta_s = singles.tile([50, ntok], f32r)
        nc.sync.dma_start(out=xta_s[:], in_=xta.ap())
        w_tiles = {}
        for nm, hnd, shp in (
            ("wqa", wqa, [50, 128]), ("wqb", wqb, [50, 128]),
            ("wka", wka, [50, 128]), ("wkb", wkb, [50, 128]),
            ("wv", wv, [50, 56]), ("w1", w1, [48, 256]),
        ):
            t = singles.tile(shp, f32r, name=nm)
            nc.sync.dma_start(out=t[:], in_=hnd.ap())
            w_tiles[nm] = t
        esel_s = singles.tile([128, 128], bf16, name="esel")
        nc.sync.dma_start(out=esel_s[:], in_=esel.ap())
        wopa_s = singles.tile([128, 48], bf16)
        nc.sync.dma_start(out=wopa_s[:], in_=wopa.ap())
        wopb_s = singles.tile([128, 48], bf16)
        nc.sync.dma_start(out=wopb_s[:], in_=wopb.ap())
        bo_s = singles.tile([48, 1], f32)
        nc.sync.dma_start(out=bo_s[:], in_=boc.ap())
        w2_s = singles.tile([128, 2, 48], f32r)
        nc.sync.dma_start(
            out=w2_s[:], in_=w2.ap().rearrange("(two p) n -> p two n", two=2)
        )
        b1_s = singles.tile([128, 2], f32)
        nc.sync.dma_start(out=b1_s[:], in_=b1c.ap())
        b2_s = singles.tile([48, 1], f32)
        nc.sync.dma_start(out=b2_s[:], in_=b2c.ap())
        ln_s = singles.tile([128, 4, 48], f32)
        lnap = lnc.ap()
        ln_bcast = bass.AP(
            tensor=lnap.tensor, offset=lnap.offset, ap=[[0, 128]] + list(lnap.ap)
        )
        nc.sync.dma_start(out=ln_s[:], in_=ln_bcast)
        eps_s = singles.tile([128, 1], f32)
        nc.vector.memset(eps_s[:], 1e-5)
        idb = singles.tile([128, 128], bf16)
        make_identity(nc, idb[:])
        idtmp = singles.tile([128, 128], f32)
        make_identity(nc, idtmp[:])
        idf = singles.tile([128, 128], f32r)
        nc.vector.tensor_copy(out=idf[:], in_=idtmp[:])

        # PSUM bank map (8 banks). Concurrent row-tiled score matmuls must
        # write 4 DIFFERENT banks (same-bank concurrent full-partition PE
        # writes fault the HW); col-tiled ctx matmuls write disjoint
        # partitions of one bank, and full-array matmuls serialize on PE,
        # so dn/ot_p can share the y1 bank.
        #   s0-s3: projections | pair-batched score strips | vt_p, v4_ps
        #   cxd:   persistent ctx(+denom-row) accumulator
        #   ft:    FFN hidden, one 256-token block at a time
        #   bscr:  B-scratch [128,512]: y2_p 0:192, y2t [0:48,192:320],
        #          h1t_p [0:48,320:448]
        #   y1:    [128,448]: y1 accum 0:192, dn 192:320, ot_p [0:48,320:448]
        ps = ctx.enter_context(tc.tile_pool(name="ps", bufs=1, space="PSUM"))
        pss = ctx.enter_context(tc.tile_pool(name="pss", bufs=1, space="PSUM"))
        # persistent ctx(+denom) accumulator: heads live at partitions
        # 32j..32j+6; the gap rows must stay exactly zero (they feed the
        # o-projection through zero weights, and stale PSUM could be
        # non-finite), so zero them once up front.
        cxd_t = pss.tile([128, 2, 128], f32, name="cxd_t")
        nc.vector.memset(cxd_t[:], 0.0)
        qk_sb = ctx.enter_context(tc.tile_pool(name="qk_sb", bufs=3))
        sm_sb = ctx.enter_context(tc.tile_pool(name="sm_sb", bufs=3))
        ln_sb = ctx.enter_context(tc.tile_pool(name="ln_sb", bufs=3))
        ff_sb = ctx.enter_context(tc.tile_pool(name="ff_sb", bufs=3))

        AX = mybir.AxisListType
        i32 = mybir.dt.int32

        def rsqrt_dve(x_ap, pfx):
            """rsqrt without ACT: magic-constant seed (DVE int ops) + one
            Newton step on Pool. x_ap is [128, CHUNK] f32 (variance; eps
            dropped — var of real tokens is O(1) here and degenerate rows
            stay finite and are discarded by the host gather)."""
            P = ln_sb
            y = P.tile([128, CHUNK], f32, tag=f"{pfx}ys", name=f"{pfx}ys")
            t1 = P.tile([128, CHUNK], f32, tag=f"{pfx}t1", name=f"{pfx}t1")
            nc.vector.tensor_scalar(
                out=y[:].bitcast(i32), in0=x_ap.bitcast(i32),
                scalar1=1, scalar2=-1,
                op0=ALU.logical_shift_right, op1=ALU.bitwise_xor,
            )
            nc.vector.tensor_scalar(
                out=y[:].bitcast(i32), in0=y[:].bitcast(i32),
                scalar1=0x5F3759E0, scalar2=None, op0=ALU.add,
            )
            nc.gpsimd.tensor_mul(t1[:], y[:], y[:])
            nc.gpsimd.tensor_mul(t1[:], t1[:], x_ap)
            nc.vector.tensor_scalar(
                out=t1[:], in0=t1[:], scalar1=-0.5, scalar2=1.5,
                op0=ALU.mult, op1=ALU.add,
            )
            nc.gpsimd.tensor_mul(y[:], y[:], t1[:])
            return y

        def ln_batch(yw_fn, yfull_ap, g_ap, b_ap, pfx, ch, out_dtype=None,
                     skip_bias=False):
            """Chunk-batched LayerNorm. yw_fn(wi) -> [128, 48] f32 AP for one
            window, yfull_ap -> [128, CHUNK, 48] f32 AP (PSUM ok). DVE-only
            (no ACT): bn_stats/bn_aggr for stats, magic-rsqrt for
            1/sqrt(var+eps). With skip_bias, returns (y-mu)*rstd*g (bias
            folded into downstream weights)."""
            od = out_dtype if out_dtype is not None else f32r
            P = ln_sb
            st6 = P.tile([128, CHUNK, 6], f32, tag=f"{pfx}st", name=f"{pfx}st")
            mv = P.tile([128, CHUNK, 2], f32, tag=f"{pfx}mv", name=f"{pfx}mv")
            for wi in range(CHUNK):
                nc.vector.bn_stats(out=st6[:, wi, :], in_=yw_fn(wi))
                nc.vector.bn_aggr(out=mv[:, wi, :], in_=st6[:, wi, :])
            va = mv[:, :, 1:2]
            var_ap = bass.AP(tensor=va.tensor, offset=va.offset,
                             ap=[va.ap[0], va.ap[1]])
            rstd = rsqrt_dve(var_ap, pfx)
            ma = mv[:, :, 0:1]
            mean_b = bass.AP(tensor=ma.tensor, offset=ma.offset,
                             ap=[ma.ap[0], ma.ap[1], [0, 48]])
            t = P.tile([128, CHUNK, 48], f32, tag=f"{pfx}t", name=f"{pfx}t")
            nc.vector.tensor_sub(t[:], yfull_ap, mean_b)
            nc.gpsimd.tensor_mul(t[:], t[:], _b(rstd[:]))
            gv = bass.AP(tensor=g_ap.tensor, offset=g_ap.offset,
                         ap=[g_ap.ap[0], [0, CHUNK], g_ap.ap[1]])
            bv = bass.AP(tensor=b_ap.tensor, offset=b_ap.offset,
                         ap=[b_ap.ap[0], [0, CHUNK], b_ap.ap[1]])
            h = P.tile([128, CHUNK, 48], od, tag=f"{pfx}h", name=f"{pfx}h")
            nc.gpsimd.tensor_mul(h[:], t[:], gv)
            if not skip_bias:
                nc.gpsimd.tensor_add(h[:], h[:], bv)
            return h

        rep_cm = tc.For_i(0, reps, 1) if reps > 1 else None
        if rep_cm is not None:
            rep_cm.__enter__()
        for ch in range(nchunk):
            t0 = ch * CHUNK * T
            NTK = CHUNK * T
            xsl = xta_s[:, t0 : t0 + NTK]

            # ---- A stage: projections + windowed attention ----
            projs = {}
            for nm, wt, tag in (
                ("qa", "wqa", "s0"), ("qb", "wqb", "s1"),
                ("ka", "wka", "s2"), ("kb", "wkb", "s3"),
            ):
                p = ps.tile([128, NTK], f32, tag=tag, name=f"p{nm}")
                nc.tensor.matmul(p[:], w_tiles[wt][:], xsl)
                t = qk_sb.tile([128, NTK], bf16, tag=nm, name=f"s{nm}")
                if nm in ("qa", "ka"):
                    nc.vector.tensor_copy(out=t[:], in_=p[:])
                else:
                    nc.scalar.copy(out=t[:], in_=p[:])
                projs[nm] = t
            qkev = projs
            vt_p = ps.tile([56, NTK], f32, tag="s0", name="vt_p")
            nc.tensor.matmul(vt_p[:], w_tiles["wv"][:], xsl)
            vt_s = qk_sb.tile([56, NTK], bf16, tag="vt", name="vt_s")
            nc.vector.tensor_copy(out=vt_s[:], in_=vt_p[:])
            # all 4 windows' v transposed up front
            v4_ps = ps.tile([128, CHUNK, 56], bf16, tag="s1", name="v4_ps")
            for wi in range(CHUNK):
                nc.tensor.transpose(
                    v4_ps[:, wi, :], vt_s[:, wi * T : wi * T + T],
                    idb[0:56, 0:56],
                )
            v4_s = sm_sb.tile([128, CHUNK, 56], bf16, tag="v", name="v4_s")
            nc.vector.tensor_copy(out=v4_s[:], in_=v4_ps[:])

            h1t_s = ff_sb.tile([48, NTK], f32r, tag="h1t", name="h1t_s")

            # y1 accumulator + per-window denominator + o-projection share
            # one bank (all written by full-array matmuls, which serialize)
            y1x = ps.tile([128, 448], f32, tag="y1", name="y1x")
            dnb = y1x[:, 192:320]
            ot_v = y1x[0:48, 320:448]
            for pair in range(CHUNK // 2):
                # scores for a 2-window pair, k-major: out[k, q] — lhsT is
                # the k strip (mask lane lands on partitions), rhs the q
                # strip (ones lane). 4 concurrent strips -> 4 banks.
                scp = [
                    ps.tile([128, 2, 2, 128], f32, tag=f"s{i}", name=f"scp{i}")
                    for i in range(4)
                ]
                if "scores" not in ablate:
                    for ws in range(2):
                        w0 = (2 * pair + ws) * T
                        for rnd, (qs, ks) in enumerate(
                            ((qkev["qa"], qkev["ka"]), (qkev["qb"], qkev["kb"]))
                        ):
                            for i in range(4):
                                nc.tensor.matmul(
                                    scp[i][:, ws, rnd, :],
                                    ks[32 * i : 32 * i + 8, w0 : w0 + T],
                                    qs[32 * i : 32 * i + 8, w0 : w0 + T],
                                    tile_position=(32 * i, 0),
                                )

                # attn_T: [128 k, strip i, win ws, round r, 128 q]; h = 4r+i
                attn_s = sm_sb.tile(
                    [128, 4, 2, 2, 128], bf16, tag="attn", name="attn_s"
                )
                for i in range(4 if "scores" not in ablate else 0):
                    nc.scalar.activation(
                        out=attn_s[:, i, :, :, :], in_=scp[i][:], func=AF.Exp
                    )

                for ws in range(2):
                    wi = 2 * pair + ws
                    w0 = wi * T
                    # ctx col-tiled (disjoint partitions per col group); v
                    # carries a ones column per head, so row 32j+6 of cxd
                    # accumulates the softmax denominator.
                    for h in range(8 if "attn_tail" not in ablate else 0):
                        r, j = divmod(h, 4)
                        nc.tensor.matmul(
                            cxd_t[32 * j : 32 * j + 7, r, :],
                            v4_s[:, wi, 7 * h : 7 * h + 7],
                            attn_s[:, j, ws, r, :],
                            tile_position=(0, 32 * j),
                        )

                    if "attn_tail" in ablate:
                        ot_s = sm_sb.tile([48, 128], f32r, tag="ots", name="ot_s")
                        nc.scalar.copy(out=ot_s[:], in_=xsl[0:48, w0 : w0 + T])
                    else:
                        cx2 = sm_sb.tile([128, 2, 128], bf16, tag="cxs", name="cx2")
                        nc.scalar.copy(out=cx2[:], in_=cxd_t[:])
                        # broadcast denom rows across their 32-block,
                        # reciprocal, normalize (denom rows normalize to ~1;
                        # Wo rows there are zero). dn done per round through
                        # one [128,128] region of the y1 bank.
                        rc_s = sm_sb.tile([128, 2, 128], bf16, tag="rc", name="rc_s")
                        for r in range(2):
                            nc.tensor.matmul(dnb, esel_s[:], cx2[:, r, :])
                            with nc.allow_low_precision(
                                reason="softmax denom in bf16"
                            ):
                                nc.vector.reciprocal(rc_s[:, r, :], dnb)
                        nr_s = sm_sb.tile([128, 2, 128], bf16, tag="nrm", name="nr_s")
                        nc.gpsimd.tensor_mul(nr_s[:], cx2[:], rc_s[:])

                        # oT = sum_r WoPad_r.T @ nr_r  (+bo on eviction)
                        nc.tensor.matmul(
                            ot_v, wopa_s[:], nr_s[:, 0, :],
                            start=True, stop=False,
                        )
                        nc.tensor.matmul(
                            ot_v, wopb_s[:], nr_s[:, 1, :],
                            start=False, stop=True,
                        )
                        ot_s = sm_sb.tile([48, 128], f32r, tag="ots", name="ot_s")
                        nc.scalar.activation(
                            out=ot_s[:], in_=ot_v, func=AF.Identity,
                            bias=bo_s[:],
                        )

                    # y1 = x + o via two transposes into one PSUM accum group
                    y1w = y1x[:, 48 * wi : 48 * wi + 48].bitcast(f32r)
                    nc.tensor.matmul(
                        y1w,
                        xsl[0:48, w0 : w0 + T],
                        idf[0:48, 0:48],
                        is_transpose=True,
                        start=True,
                        stop=False,
                    )
                    nc.tensor.matmul(
                        y1w,
                        ot_s[:],
                        idf[0:48, 0:48],
                        is_transpose=True,
                        start=False,
                        stop=True,
                    )

            # ---- B stage: LN1, FFN, residual 2, LN2, store ----
            bscr = ps.tile([128, 512], f32, tag="bscr", name="bscr")
            # ln1_b is folded into b1/b2 host-side (skip_bias)
            y1full = y1x[:, 0:192]
            y1full = bass.AP(tensor=y1full.tensor, offset=y1full.offset,
                             ap=[y1full.ap[0], [48, CHUNK], [1, 48]])
            h1_s = ln_batch(
                lambda wi: y1x[:, 48 * wi : 48 * wi + 48],
                y1full,
                ln_s[:, 0, :], ln_s[:, 1, :], "ln1", ch, skip_bias=True,
            )
            h1t_pv = bscr[0:48, 320:448].bitcast(f32r)
            for wi in range(CHUNK):
                w0 = wi * T
                nc.tensor.transpose(h1t_pv, h1_s[:, wi, :], idf[:])
                if wi % 2 == 0:
                    nc.vector.tensor_copy(
                        out=h1t_s[:, w0 : w0 + T], in_=h1t_pv
                    )
                else:
                    nc.scalar.copy(out=h1t_s[:, w0 : w0 + T], in_=h1t_pv)

            # FFN in two 256-token blocks through one PSUM bank
            y2t_s = ff_sb.tile([48, NTK], f32r, tag="y2ts", name="y2t_s")
            y2t_pv = bscr[0:48, 192:320]
            for tb in range(2):
                c0 = tb * 256
                ft_p = ps.tile([128, 2, 256], f32, tag="ft", name="ft_p")
                fr = ff_sb.tile([128, 2, 256], f32r, tag="fr", name="fr")
                for half in range(2):
                    nc.tensor.matmul(
                        ft_p[:, half, :],
                        w_tiles["w1"][:, 128 * half : 128 * half + 128],
                        h1t_s[:, c0 : c0 + 256],
                    )
                    nc.vector.tensor_scalar(
                        out=fr[:, half, :], in0=ft_p[:, half, :],
                        scalar1=b1_s[:, half : half + 1], scalar2=0.0,
                        op0=ALU.add, op1=ALU.max,
                    )
                for qb in range(2):
                    q0 = qb * 128
                    nc.tensor.matmul(
                        y2t_pv, w2_s[:, 0, :], fr[:, 0, q0 : q0 + 128],
                        start=True, stop=False,
                    )
                    nc.tensor.matmul(
                        y2t_pv, w2_s[:, 1, :], fr[:, 1, q0 : q0 + 128],
                        start=False, stop=True,
                    )
                    nc.vector.tensor_scalar_add(
                        y2t_s[:, c0 + q0 : c0 + q0 + 128], y2t_pv, b2_s[:]
                    )

            # y2 = h1 + ff: both live channel-major (h1t_s, y2t_s) —
            # transpose both into one PSUM accumulation group per window
            for wi in range(CHUNK):
                w0 = wi * T
                y2w = bscr[:, 48 * wi : 48 * wi + 48].bitcast(f32r)
                nc.tensor.matmul(
                    y2w, y2t_s[:, w0 : w0 + T], idf[0:48, 0:48],
                    is_transpose=True, start=True, stop=False,
                )
                nc.tensor.matmul(
                    y2w, h1t_s[:, w0 : w0 + T], idf[0:48, 0:48],
                    is_transpose=True, start=False, stop=True,
                )
            y2full = bscr[:, 0:192]
            y2full = bass.AP(tensor=y2full.tensor, offset=y2full.offset,
                             ap=[y2full.ap[0], [48, CHUNK], [1, 48]])
            h2 = ln_batch(
                lambda wi: bscr[:, 48 * wi : 48 * wi + 48],
                y2full,
                ln_s[:, 2, :], ln_s[:, 3, :], "ln2", ch, out_dtype=f32,
            )
            if "dma" in ablate:
                continue
            nc.sync.dma_start(
                out=out.ap()[ch * CHUNK : (ch + 1) * CHUNK, :, :].rearrange(
                    "w t c -> t w c"
                ),
                in_=h2[:].bitcast(f32),
            )
        if rep_cm is not None:
            rep_cm.__exit__(None, None, None)

    nc.compile()
    return nc


def _prep_host(voxel_features, voxel_coords, Wq, bq, Wk, bk, Wv, bv, Wo, bo,
               ln1_g, ln1_b, W1, b1, W2, b2, ln2_g, ln2_b, wpc=WPC,
               ncores=NCORES):
    f32 = np.float32
    vc = np.asarray(voxel_coords)
    b, z, y, x = vc[:, 0], vc[:, 1], vc[:, 2], vc[:, 3]
    win = ((b * (GZ // WZ) + z // WZ) * (GY // WY) + y // WY) * (GX // WX) + x // WX
    slot = (z % WZ) * (WY * WX) + (y % WY) * WX + (x % WX)
    win = np.asarray(win, np.int64)
    slot = np.asarray(slot, np.int64)

    nwp = ncores * wpc
    xta = np.zeros((nwp, 50, T), f32)
    xta[:, 48, :] = 1.0
    xta[win, :48, slot] = np.asarray(voxel_features, f32)
    mask = np.full((nwp, T), MASKVAL, f32)
    occupied = np.zeros(nwp, bool)
    occupied[win] = True
    mask[~occupied] = 0.0
    mask[win, slot] = 0.0
    xta[:, 49, :] = mask

    s = f32(1.0 / np.sqrt(HD))
    Wq_s = np.asarray(Wq, f32) * s
    bq_s = np.asarray(bq, f32) * s
    bf = ml_dtypes.bfloat16

    def qk_pack(W, bvec, mask_lane):
        A = np.zeros((2, 50, 128), f32)
        for h in range(8):
            half, i = divmod(h, 4)
            A[half, :48, 32 * i : 32 * i + 6] = W[:, 6 * h : 6 * h + 6]
            A[half, 48, 32 * i : 32 * i + 6] = bvec[6 * h : 6 * h + 6]
            A[half, 49 if mask_lane else 48, 32 * i + 6] = 1.0
        return A[0], A[1]

    wqa_a, wqb_a = qk_pack(Wq_s, bq_s, mask_lane=False)
    wka_a, wkb_a = qk_pack(np.asarray(Wk, f32), np.asarray(bk, f32),
                           mask_lane=True)
    # v packed [50, 56]: cols 7h+d = Wv col 6h+d, col 7h+6 = ones-row pick
    # (row 48 of xta is the constant-1 row) so the ctx matmul's 7th output
    # row per head is the softmax denominator.
    wv_a = np.zeros((50, 56), f32)
    Wv_f = np.asarray(Wv, f32)
    bv_f = np.asarray(bv, f32)
    for h in range(8):
        wv_a[:48, 7 * h : 7 * h + 6] = Wv_f[:, 6 * h : 6 * h + 6]
        wv_a[48, 7 * h : 7 * h + 6] = bv_f[6 * h : 6 * h + 6]
        wv_a[48, 7 * h + 6] = 1.0
    # esel[32j+6, 32j:32j+32] = 1: expands each denom row across its whole
    # 32-block (gap rows included, keeping reciprocal finite)
    esel_a = np.zeros((128, 128), np.float32)
    for j in range(4):
        esel_a[32 * j + 6, 32 * j : 32 * j + 32] = 1.0
    wop = np.zeros((2, 128, 48), f32)
    for h in range(8):
        r, j = divmod(h, 4)
        wop[r, 32 * j : 32 * j + 6, :] = np.asarray(Wo, f32)[6 * h : 6 * h + 6, :]
    w1_a = np.ascontiguousarray(np.asarray(W1, f32))
    # LN1's bias is skipped on-device (h1_nob = (y-mu)*rstd*g); fold it into
    # the FFN biases: b1' = b1 + ln1_b@W1, b2' = b2 + ln1_b.
    b1_f = np.asarray(b1, f32) + np.asarray(ln1_b, f32) @ w1_a
    b1c_a = np.stack([b1_f[:128], b1_f[128:]], 1)
    b2c_a = (np.asarray(b2, f32) + np.asarray(ln1_b, f32)).reshape(48, 1)
    ln_a = np.stack([ln1_g, ln1_b, ln2_g, ln2_b]).astype(f32)

    weights = dict(
        wqa=wqa_a, wqb=wqb_a, wka=wka_a, wkb=wkb_a, wv=wv_a,
        esel=esel_a.astype(bf),
        wopa=wop[0].astype(bf), wopb=wop[1].astype(bf),
        boc=np.asarray(bo, f32).reshape(48, 1),
        w1=w1_a, w2=np.ascontiguousarray(np.asarray(W2, f32)),
        b1c=np.ascontiguousarray(b1c_a), b2c=b2c_a,
        lnc=np.ascontiguousarray(ln_a),
    )
    in_maps = []
    for c in range(ncores):
        m = dict(weights)
        sh = xta[c * wpc : (c + 1) * wpc]  # [wpc, 50, T]
        m["xta"] = np.ascontiguousarray(
            sh.transpose(1, 0, 2).reshape(50, wpc * T)
        )
        in_maps.append(m)
    return in_maps, win, slot


def kernel(**inputs):
    key = ("full", WPC)
    if key not in _CACHE:
        _CACHE[key] = _build_bass(WPC)
    nc = _CACHE[key]
    in_maps, win, slot = _prep_host(**inputs)
    from concourse import bass_utils

    r = bass_utils.run_bass_kernel_spmd(
        nc, in_maps, core_ids=list(range(NCORES))
    )
    full = np.concatenate([r.results[c]["out"] for c in range(NCORES)], 0)
    return full[win, slot].astype(np.float32)



# revision 61
# speedup vs baseline: 1.3414x; 1.3414x over previous
"""Windowed sparse point-transformer layer on 8 Trainium2 NeuronCores.

Strategy (spec sharding_hint): windows are independent, so the host
scatters voxels into dense per-window buffers (the "all-to-all keyed by
window id" done as a host-side relayout), shards 900->928 windows across
8 cores (116 each), and each core runs a dense windowed attention+FFN
Bass kernel. Weights are replicated. The host gathers per-window outputs
back to the sparse voxel list.

Device kernel per window (T=128 slots, C=48 channels, 8 heads x 6 dim):
  - q/k projections produce CHANNEL-major qT/kT with heads padded onto
    32-partition strips so the 8 score matmuls use PE row tiling
    (K=8 per head: 6 data lanes + a ones/mask lane that applies the
    -1e5 padding mask during the matmul itself; bias is folded in via
    the input's ones row).
  - softmax (q-major): exp on ACT with fused row-sum accumulation,
    reciprocal + per-head normalize on DVE (bf16), attn transposed on
    PE, ctx matmuls against bf16 v accumulate fp32 in PSUM.
  - residual 1: x.T is PE-transposed straight into the same PSUM
    accumulation group as the output projection (ctx @ Wo + bo).
  - FFN batched across a 4-window chunk (N=512) in float32r so the PE
    runs at full rate; b1/b2 ride the activation bias port during PSUM
    eviction. LayerNorms run token-major on DVE/ACT (bn_stats/bn_aggr).

PSUM is 8 banks; tile tags are assigned so the live set maps onto 8
bank-sized slots (s0-s3 shared by projections/scores/FFN, at, misc,
cx, y1).
"""

import numpy as np
import ml_dtypes
from contextlib import ExitStack

GX, GY, GZ = 120, 120, 8
WX, WY, WZ = 8, 8, 2
T = 128
C = 48
H = 8
HD = 6
FF = 256
N = 80000
NW = (GX // WX) * (GY // WY) * (GZ // WZ)  # 900
NCORES = 8
WPC = 116  # windows per core (900 padded to 928 = 8*116)
CHUNK = 4  # windows per projection/FFN batch (N=512 tokens)
MASKVAL = -1e5

_CACHE = {}


def _build_bass(wpc, reps=1, ablate=()):
    import concourse.bass as bass
    import concourse.tile as tile
    import concourse.mybir as mybir
    from concourse import bacc
    from concourse.masks import make_identity

    f32 = mybir.dt.float32
    f32r = mybir.dt.float32r
    bf16 = mybir.dt.bfloat16
    AF = mybir.ActivationFunctionType
    ALU = mybir.AluOpType

    nc = bacc.Bacc("TRN2", target_bir_lowering=False, debug=False)
    ntok = wpc * T

    xta = nc.dram_tensor("xta", [50, ntok], f32r, kind="ExternalInput")
    wqa = nc.dram_tensor("wqa", [50, 128], f32r, kind="ExternalInput")
    wqb = nc.dram_tensor("wqb", [50, 128], f32r, kind="ExternalInput")
    wka = nc.dram_tensor("wka", [50, 128], f32r, kind="ExternalInput")
    wkb = nc.dram_tensor("wkb", [50, 128], f32r, kind="ExternalInput")
    wv = nc.dram_tensor("wv", [50, 56], f32r, kind="ExternalInput")
    esel = nc.dram_tensor("esel", [128, 128], bf16, kind="ExternalInput")
    wopa = nc.dram_tensor("wopa", [128, 48], bf16, kind="ExternalInput")
    wopb = nc.dram_tensor("wopb", [128, 48], bf16, kind="ExternalInput")
    boc = nc.dram_tensor("boc", [48, 1], f32, kind="ExternalInput")
    w1 = nc.dram_tensor("w1", [49, 256], f32r, kind="ExternalInput")
    w2 = nc.dram_tensor("w2", [256, 48], f32r, kind="ExternalInput")
    b2c = nc.dram_tensor("b2c", [48, 1], f32, kind="ExternalInput")
    lnc = nc.dram_tensor("lnc", [4, 48], f32, kind="ExternalInput")
    out = nc.dram_tensor("out", [wpc, T, C], f32, kind="ExternalOutput")

    nchunk = wpc // CHUNK
    assert wpc % CHUNK == 0

    def _bn(col, n):
        return bass.AP(tensor=col.tensor, offset=col.offset,
                       ap=[col.ap[0], [0, n]])

    def _b(ap3, n=None):
        # broadcast a [128, k] AP along a new trailing free dim of size 48
        return bass.AP(tensor=ap3.tensor, offset=ap3.offset,
                       ap=list(ap3.ap) + [[0, 48]])

    def ln_apply(pool, y_ap, g_ap, b_ap, eps_s, out_dtype, tagp):
        """LayerNorm over the free dim (C=48) of token-major y_ap."""
        mv = pool.tile([128, 2], f32, tag=f"{tagp}mv", name=f"{tagp}mv")
        st6 = pool.tile([128, 6], f32, tag=f"{tagp}st", name=f"{tagp}st")
        nc.vector.bn_stats(out=st6[:], in_=y_ap)
        nc.vector.bn_aggr(out=mv[:], in_=st6[:])
        rstd = pool.tile([128, 1], f32, tag=f"{tagp}rs", name=f"{tagp}rs")
        nc.scalar.activation(
            out=rstd[:], in_=mv[:, 1:2], func=AF.Ln, bias=eps_s[:]
        )
        nc.scalar.activation(
            out=rstd[:], in_=rstd[:], func=AF.Exp, scale=-0.5
        )
        yc = pool.tile([128, 48], f32, tag=f"{tagp}yc", name=f"{tagp}yc")
        nc.vector.tensor_scalar_sub(yc[:], y_ap, mv[:, 0:1])
        ho = pool.tile([128, 48], out_dtype, tag=f"{tagp}ho", name=f"{tagp}ho")
        nc.vector.scalar_tensor_tensor(
            out=ho[:], in0=yc[:], scalar=rstd[:], in1=g_ap,
            op0=ALU.mult, op1=ALU.mult,
        )
        nc.vector.tensor_add(ho[:], ho[:], b_ap)
        return ho

    with tile.TileContext(nc) as tc, ExitStack() as ctx:
        singles = ctx.enter_context(tc.tile_pool(name="singles", bufs=1))

        xta_s = singles.tile([50, ntok], f32r)
        nc.sync.dma_start(out=xta_s[:], in_=xta.ap())
        w_tiles = {}
        for nm, hnd, shp in (
            ("wqa", wqa, [50, 128]), ("wqb", wqb, [50, 128]),
            ("wka", wka, [50, 128]), ("wkb", wkb, [50, 128]),
            ("wv", wv, [50, 56]), ("w1", w1, [49, 256]),
        ):
            t = singles.tile(shp, f32r, name=nm)
            nc.sync.dma_start(out=t[:], in_=hnd.ap())
            w_tiles[nm] = t
        esel_s = singles.tile([128, 128], bf16, name="esel")
        nc.sync.dma_start(out=esel_s[:], in_=esel.ap())
        wopa_s = singles.tile([128, 48], bf16)
        nc.sync.dma_start(out=wopa_s[:], in_=wopa.ap())
        wopb_s = singles.tile([128, 48], bf16)
        nc.sync.dma_start(out=wopb_s[:], in_=wopb.ap())
        bo_s = singles.tile([48, 1], f32)
        nc.sync.dma_start(out=bo_s[:], in_=boc.ap())
        w2_s = singles.tile([128, 2, 48], f32r)
        nc.sync.dma_start(
            out=w2_s[:], in_=w2.ap().rearrange("(two p) n -> p two n", two=2)
        )
        b2_s = singles.tile([48, 1], f32)
        nc.sync.dma_start(out=b2_s[:], in_=b2c.ap())
        ln_s = singles.tile([128, 4, 48], f32)
        lnap = lnc.ap()
        ln_bcast = bass.AP(
            tensor=lnap.tensor, offset=lnap.offset, ap=[[0, 128]] + list(lnap.ap)
        )
        nc.sync.dma_start(out=ln_s[:], in_=ln_bcast)
        eps_s = singles.tile([128, 1], f32)
        nc.vector.memset(eps_s[:], 1e-5)
        idb = singles.tile([128, 128], bf16)
        make_identity(nc, idb[:])
        idtmp = singles.tile([128, 128], f32)
        make_identity(nc, idtmp[:])
        idf = singles.tile([128, 128], f32r)
        nc.vector.tensor_copy(out=idf[:], in_=idtmp[:])

        # PSUM bank map (8 banks). Concurrent row-tiled score matmuls must
        # write 4 DIFFERENT banks (same-bank concurrent full-partition PE
        # writes fault the HW); col-tiled ctx matmuls write disjoint
        # partitions of one bank, and full-array matmuls serialize on PE,
        # so dn/ot_p can share the y1 bank.
        #   s0-s3: projections | pair-batched score strips | vt_p, v4_ps
        #   cxd:   persistent ctx(+denom-row) accumulator
        #   ft:    FFN hidden, one 256-token block at a time
        #   bscr:  B-scratch [128,512]: y2_p 0:192, y2t [0:48,192:320],
        #          h1t_p [0:48,320:448]
        #   y1:    [128,448]: y1 accum 0:192, dn 192:320, ot_p [0:48,320:448]
        ps = ctx.enter_context(tc.tile_pool(name="ps", bufs=1, space="PSUM"))
        pss = ctx.enter_context(tc.tile_pool(name="pss", bufs=1, space="PSUM"))
        # persistent ctx(+denom) accumulator, double-slotted by window
        # parity so window wi+1's ctx matmuls don't wait on wi's eviction.
        # Heads live at partitions 32j..32j+6; the gap rows must stay
        # exactly zero (they feed the o-projection through zero weights,
        # and stale PSUM could be non-finite), so zero them once up front.
        cxd_t = pss.tile([128, 2, 2, 128], f32, name="cxd_t")
        nc.vector.memset(cxd_t[:], 0.0)
        qk_sb = ctx.enter_context(tc.tile_pool(name="qk_sb", bufs=4))
        sm_sb = ctx.enter_context(tc.tile_pool(name="sm_sb", bufs=6))
        ln_sb = ctx.enter_context(tc.tile_pool(name="ln_sb", bufs=4))
        ff_sb = ctx.enter_context(tc.tile_pool(name="ff_sb", bufs=4))

        AX = mybir.AxisListType
        i32 = mybir.dt.int32

        def rsqrt_dve(x_ap, pfx):
            """rsqrt without ACT: magic-constant seed (DVE int ops) + one
            Newton step on Pool. x_ap is [128, CHUNK] f32 (variance; eps
            dropped — var of real tokens is O(1) here and degenerate rows
            stay finite and are discarded by the host gather)."""
            P = ln_sb
            y = P.tile([128, CHUNK], f32, tag=f"{pfx}ys", name=f"{pfx}ys")
            t1 = P.tile([128, CHUNK], f32, tag=f"{pfx}t1", name=f"{pfx}t1")
            nc.vector.tensor_scalar(
                out=y[:].bitcast(i32), in0=x_ap.bitcast(i32),
                scalar1=1, scalar2=-1,
                op0=ALU.logical_shift_right, op1=ALU.bitwise_xor,
            )
            nc.vector.tensor_scalar(
                out=y[:].bitcast(i32), in0=y[:].bitcast(i32),
                scalar1=0x5F3759E0, scalar2=None, op0=ALU.add,
            )
            nc.gpsimd.tensor_mul(t1[:], y[:], y[:])
            nc.gpsimd.tensor_mul(t1[:], t1[:], x_ap)
            nc.vector.tensor_scalar(
                out=t1[:], in0=t1[:], scalar1=-0.5, scalar2=1.5,
                op0=ALU.mult, op1=ALU.add,
            )
            nc.gpsimd.tensor_mul(y[:], y[:], t1[:])
            return y

        def ln_batch(yw_fn, yfull_ap, g_ap, b_ap, pfx, ch, out_dtype=None,
                     skip_bias=False):
            """Chunk-batched LayerNorm. yw_fn(wi) -> [128, 48] f32 AP for one
            window, yfull_ap -> [128, CHUNK, 48] f32 AP (PSUM ok). DVE-only
            (no ACT): bn_stats/bn_aggr for stats, magic-rsqrt for
            1/sqrt(var+eps). With skip_bias, returns (y-mu)*rstd*g (bias
            folded into downstream weights)."""
            od = out_dtype if out_dtype is not None else f32r
            P = ln_sb
            st6 = P.tile([128, CHUNK, 6], f32, tag=f"{pfx}st", name=f"{pfx}st")
            mv = P.tile([128, CHUNK, 2], f32, tag=f"{pfx}mv", name=f"{pfx}mv")
            for wi in range(CHUNK):
                nc.vector.bn_stats(out=st6[:, wi, :], in_=yw_fn(wi))
                nc.vector.bn_aggr(out=mv[:, wi, :], in_=st6[:, wi, :])
            va = mv[:, :, 1:2]
            var_ap = bass.AP(tensor=va.tensor, offset=va.offset,
                             ap=[va.ap[0], va.ap[1]])
            rstd = rsqrt_dve(var_ap, pfx)
            ma = mv[:, :, 0:1]
            mean_b = bass.AP(tensor=ma.tensor, offset=ma.offset,
                             ap=[ma.ap[0], ma.ap[1], [0, 48]])
            t = P.tile([128, CHUNK, 48], f32, tag=f"{pfx}t", name=f"{pfx}t")
            nc.vector.tensor_sub(t[:], yfull_ap, mean_b)
            nc.gpsimd.tensor_mul(t[:], t[:], _b(rstd[:]))
            gv = bass.AP(tensor=g_ap.tensor, offset=g_ap.offset,
                         ap=[g_ap.ap[0], [0, CHUNK], g_ap.ap[1]])
            bv = bass.AP(tensor=b_ap.tensor, offset=b_ap.offset,
                         ap=[b_ap.ap[0], [0, CHUNK], b_ap.ap[1]])
            if skip_bias:
                # LN1: append a constant-1 column so the downstream FFN
                # matmul can carry b1 as a 49th weight row
                h = P.tile([128, CHUNK, 49], od, tag=f"{pfx}h", name=f"{pfx}h")
                nc.gpsimd.tensor_mul(h[:, :, 0:48], t[:], gv)
                nc.gpsimd.memset(h[:, :, 48:49].bitcast(f32), 1.0)
            else:
                h = P.tile([128, CHUNK, 48], od, tag=f"{pfx}h", name=f"{pfx}h")
                nc.gpsimd.tensor_mul(h[:], t[:], gv)
                nc.gpsimd.tensor_add(h[:], h[:], bv)
            return h

        NTK = CHUNK * T

        def stage_a(ch):
            t0 = ch * CHUNK * T
            xsl = xta_s[:, t0 : t0 + NTK]

            # ---- A stage: projections + windowed attention ----
            projs = {}
            for nm, wt, tag in (
                ("qa", "wqa", "s0"), ("qb", "wqb", "s1"),
                ("ka", "wka", "s2"), ("kb", "wkb", "s3"),
            ):
                p = ps.tile([128, NTK], f32, tag=tag, name=f"p{nm}")
                nc.tensor.matmul(p[:], w_tiles[wt][:], xsl)
                t = qk_sb.tile([128, NTK], bf16, tag=nm, name=f"s{nm}")
                if nm in ("qa", "ka"):
                    nc.vector.tensor_copy(out=t[:], in_=p[:])
                else:
                    nc.scalar.copy(out=t[:], in_=p[:])
                projs[nm] = t
            qkev = projs
            vt_p = ps.tile([56, NTK], f32, tag="s0", name="vt_p")
            nc.tensor.matmul(vt_p[:], w_tiles["wv"][:], xsl)
            vt_s = qk_sb.tile([56, NTK], bf16, tag="vt", name="vt_s")
            nc.vector.tensor_copy(out=vt_s[:], in_=vt_p[:])
            # all 4 windows' v transposed up front
            v4_ps = ps.tile([128, CHUNK, 56], bf16, tag="s1", name="v4_ps")
            for wi in range(CHUNK):
                nc.tensor.transpose(
                    v4_ps[:, wi, :], vt_s[:, wi * T : wi * T + T],
                    idb[0:56, 0:56],
                )
            v4_s = sm_sb.tile([128, CHUNK, 56], bf16, tag="v", name="v4_s")
            nc.vector.tensor_copy(out=v4_s[:], in_=v4_ps[:])

            # y1 accumulator + per-window denominator + o-projection share
            # one bank (all written by full-array matmuls, which serialize)
            y1x = ps.tile([128, 448], f32, tag="y1", name="y1x")
            dnb = y1x[:, 192:320]
            ot_v = y1x[0:48, 320:448]
            for pair in range(CHUNK // 2):
                # scores for a 2-window pair, k-major: out[k, q] — lhsT is
                # the k strip (mask lane lands on partitions), rhs the q
                # strip (ones lane). 4 concurrent strips -> 4 banks.
                scp = [
                    ps.tile([128, 2, 2, 128], f32, tag=f"s{i}", name=f"scp{i}")
                    for i in range(4)
                ]
                if "scores" not in ablate:
                    for ws in range(2):
                        w0 = (2 * pair + ws) * T
                        for rnd, (qs, ks) in enumerate(
                            ((qkev["qa"], qkev["ka"]), (qkev["qb"], qkev["kb"]))
                        ):
                            for i in range(4):
                                nc.tensor.matmul(
                                    scp[i][:, ws, rnd, :],
                                    ks[32 * i : 32 * i + 8, w0 : w0 + T],
                                    qs[32 * i : 32 * i + 8, w0 : w0 + T],
                                    tile_position=(32 * i, 0),
                                )

                # attn_T: [128 k, strip i, win ws, round r, 128 q]; h = 4r+i
                attn_s = sm_sb.tile(
                    [128, 4, 2, 2, 128], bf16, tag="attn", name="attn_s"
                )
                for i in range(4 if "scores" not in ablate else 0):
                    nc.scalar.activation(
                        out=attn_s[:, i, :, :, :], in_=scp[i][:], func=AF.Exp
                    )

                for ws in range(2):
                    wi = 2 * pair + ws
                    w0 = wi * T
                    # ctx col-tiled (disjoint partitions per col group); v
                    # carries a ones column per head, so row 32j+6 of cxd
                    # accumulates the softmax denominator.
                    for h in range(8 if "attn_tail" not in ablate else 0):
                        r, j = divmod(h, 4)
                        nc.tensor.matmul(
                            cxd_t[32 * j : 32 * j + 7, wi % 2, r, :],
                            v4_s[:, wi, 7 * h : 7 * h + 7],
                            attn_s[:, j, ws, r, :],
                            tile_position=(0, 32 * j),
                        )

                    if "attn_tail" in ablate:
                        ot_s = sm_sb.tile([48, 128], f32r, tag="ots", name="ot_s")
                        nc.scalar.copy(out=ot_s[:], in_=xsl[0:48, w0 : w0 + T])
                    else:
                        cx2 = sm_sb.tile([128, 2, 128], bf16, tag="cxs", name="cx2")
                        nc.scalar.copy(out=cx2[:], in_=cxd_t[:, wi % 2, :, :])
                        # broadcast denom rows across their 32-block,
                        # reciprocal, normalize (denom rows normalize to ~1;
                        # Wo rows there are zero). dn done per round through
                        # one [128,128] region of the y1 bank.
                        rc_s = sm_sb.tile([128, 2, 128], bf16, tag="rc", name="rc_s")
                        for r in range(2):
                            nc.tensor.matmul(dnb, esel_s[:], cx2[:, r, :])
                            with nc.allow_low_precision(
                                reason="softmax denom in bf16"
                            ):
                                nc.vector.reciprocal(rc_s[:, r, :], dnb)
                        nr_s = sm_sb.tile([128, 2, 128], bf16, tag="nrm", name="nr_s")
                        nc.gpsimd.tensor_mul(nr_s[:], cx2[:], rc_s[:])

                        # oT = sum_r WoPad_r.T @ nr_r  (+bo on eviction)
                        nc.tensor.matmul(
                            ot_v, wopa_s[:], nr_s[:, 0, :],
                            start=True, stop=False,
                        )
                        nc.tensor.matmul(
                            ot_v, wopb_s[:], nr_s[:, 1, :],
                            start=False, stop=True,
                        )
                        ot_s = sm_sb.tile([48, 128], f32r, tag="ots", name="ot_s")
                        nc.scalar.activation(
                            out=ot_s[:], in_=ot_v, func=AF.Identity,
                            bias=bo_s[:],
                        )

                    # y1 = x + o via two transposes into one PSUM accum group
                    y1w = y1x[:, 48 * wi : 48 * wi + 48].bitcast(f32r)
                    nc.tensor.matmul(
                        y1w,
                        xsl[0:48, w0 : w0 + T],
                        idf[0:48, 0:48],
                        is_transpose=True,
                        start=True,
                        stop=False,
                    )
                    nc.tensor.matmul(
                        y1w,
                        ot_s[:],
                        idf[0:48, 0:48],
                        is_transpose=True,
                        start=False,
                        stop=True,
                    )

            # LN1 closes stage A (y1x then has no cross-stage readers);
            # ln1_b is folded into b1/b2 host-side (skip_bias)
            y1full = y1x[:, 0:192]
            y1full = bass.AP(tensor=y1full.tensor, offset=y1full.offset,
                             ap=[y1full.ap[0], [48, CHUNK], [1, 48]])
            h1_s = ln_batch(
                lambda wi: y1x[:, 48 * wi : 48 * wi + 48],
                y1full,
                ln_s[:, 0, :], ln_s[:, 1, :], "ln1", ch, skip_bias=True,
            )
            return h1_s

        def stage_b(ch, h1_s):
            # ---- B stage: FFN, residual 2, LN2, store (banks: ft, bscr) --
            xsl = None  # unused
            bscr = ps.tile([128, 512], f32, tag="bscr", name="bscr")
            h1t_s = ff_sb.tile([49, NTK], f32r, tag="h1t", name="h1t_s")
            h1t_pv = bscr[0:49, 320:448].bitcast(f32r)
            for wi in range(CHUNK):
                w0 = wi * T
                nc.tensor.transpose(h1t_pv, h1_s[:, wi, :], idf[:])
                if wi % 2 == 0:
                    nc.vector.tensor_copy(
                        out=h1t_s[:, w0 : w0 + T], in_=h1t_pv
                    )
                else:
                    nc.scalar.copy(out=h1t_s[:, w0 : w0 + T], in_=h1t_pv)

            # FFN in two 256-token blocks through one PSUM bank (b1 rides
            # the 49th row of w1 against h1t's ones row)
            y2t_s = ff_sb.tile([48, NTK], f32r, tag="y2ts", name="y2t_s")
            y2t_pv = bscr[0:48, 192:320]
            for tb in range(2):
                c0 = tb * 256
                ft_p = ps.tile([128, 2, 256], f32, tag="ft", name="ft_p")
                fr = ff_sb.tile([128, 2, 256], f32r, tag="fr", name="fr")
                for half in range(2):
                    nc.tensor.matmul(
                        ft_p[:, half, :],
                        w_tiles["w1"][:, 128 * half : 128 * half + 128],
                        h1t_s[:, c0 : c0 + 256],
                    )
                nc.vector.tensor_scalar(
                    out=fr[:], in0=ft_p[:],
                    scalar1=0.0, scalar2=None, op0=ALU.max,
                )
                for qb in range(2):
                    q0 = qb * 128
                    nc.tensor.matmul(
                        y2t_pv, w2_s[:, 0, :], fr[:, 0, q0 : q0 + 128],
                        start=True, stop=False,
                    )
                    nc.tensor.matmul(
                        y2t_pv, w2_s[:, 1, :], fr[:, 1, q0 : q0 + 128],
                        start=False, stop=True,
                    )
                    nc.vector.tensor_scalar_add(
                        y2t_s[:, c0 + q0 : c0 + q0 + 128], y2t_pv, b2_s[:]
                    )

            # y2 = h1 + ff: both live channel-major (h1t_s, y2t_s) —
            # transpose both into one PSUM accumulation group per window
            for wi in range(CHUNK):
                w0 = wi * T
                y2w = bscr[:, 48 * wi : 48 * wi + 48].bitcast(f32r)
                nc.tensor.matmul(
                    y2w, y2t_s[:, w0 : w0 + T], idf[0:48, 0:48],
                    is_transpose=True, start=True, stop=False,
                )
                nc.tensor.matmul(
                    y2w, h1t_s[0:48, w0 : w0 + T], idf[0:48, 0:48],
                    is_transpose=True, start=False, stop=True,
                )
            y2full = bscr[:, 0:192]
            y2full = bass.AP(tensor=y2full.tensor, offset=y2full.offset,
                             ap=[y2full.ap[0], [48, CHUNK], [1, 48]])
            h2 = ln_batch(
                lambda wi: bscr[:, 48 * wi : 48 * wi + 48],
                y2full,
                ln_s[:, 2, :], ln_s[:, 3, :], "ln2", ch, out_dtype=f32,
            )
            if "dma" in ablate:
                return
            nc.sync.dma_start(
                out=out.ap()[ch * CHUNK : (ch + 1) * CHUNK, :, :].rearrange(
                    "w t c -> t w c"
                ),
                in_=h2[:].bitcast(f32),
            )

        # Software-pipelined emission: A(ch+1) is emitted before B(ch) so
        # the scheduler prioritizes the attention ring of the next chunk
        # over the FFN/LN tail of the current one (B fills engine gaps).
        rep_cm = tc.For_i(0, reps, 1) if reps > 1 else None
        if rep_cm is not None:
            rep_cm.__enter__()
        prev = stage_a(0)
        for ch in range(1, nchunk):
            cur = stage_a(ch)
            stage_b(ch - 1, prev)
            prev = cur
        stage_b(nchunk - 1, prev)
        if rep_cm is not None:
            rep_cm.__exit__(None, None, None)

    nc.compile()
    return nc


def _prep_host(voxel_features, voxel_coords, Wq, bq, Wk, bk, Wv, bv, Wo, bo,
               ln1_g, ln1_b, W1, b1, W2, b2, ln2_g, ln2_b, wpc=WPC,
               ncores=NCORES):
    f32 = np.float32
    vc = np.asarray(voxel_coords)
    b, z, y, x = vc[:, 0], vc[:, 1], vc[:, 2], vc[:, 3]
    win = ((b * (GZ // WZ) + z // WZ) * (GY // WY) + y // WY) * (GX // WX) + x // WX
    slot = (z % WZ) * (WY * WX) + (y % WY) * WX + (x % WX)
    win = np.asarray(win, np.int64)
    slot = np.asarray(slot, np.int64)

    nwp = ncores * wpc
    xta = np.zeros((nwp, 50, T), f32)
    xta[:, 48, :] = 1.0
    xta[win, :48, slot] = np.asarray(voxel_features, f32)
    mask = np.full((nwp, T), MASKVAL, f32)
    occupied = np.zeros(nwp, bool)
    occupied[win] = True
    mask[~occupied] = 0.0
    mask[win, slot] = 0.0
    xta[:, 49, :] = mask

    s = f32(1.0 / np.sqrt(HD))
    Wq_s = np.asarray(Wq, f32) * s
    bq_s = np.asarray(bq, f32) * s
    bf = ml_dtypes.bfloat16

    def qk_pack(W, bvec, mask_lane):
        A = np.zeros((2, 50, 128), f32)
        for h in range(8):
            half, i = divmod(h, 4)
            A[half, :48, 32 * i : 32 * i + 6] = W[:, 6 * h : 6 * h + 6]
            A[half, 48, 32 * i : 32 * i + 6] = bvec[6 * h : 6 * h + 6]
            A[half, 49 if mask_lane else 48, 32 * i + 6] = 1.0
        return A[0], A[1]

    wqa_a, wqb_a = qk_pack(Wq_s, bq_s, mask_lane=False)
    wka_a, wkb_a = qk_pack(np.asarray(Wk, f32), np.asarray(bk, f32),
                           mask_lane=True)
    # v packed [50, 56]: cols 7h+d = Wv col 6h+d, col 7h+6 = ones-row pick
    # (row 48 of xta is the constant-1 row) so the ctx matmul's 7th output
    # row per head is the softmax denominator.
    wv_a = np.zeros((50, 56), f32)
    Wv_f = np.asarray(Wv, f32)
    bv_f = np.asarray(bv, f32)
    for h in range(8):
        wv_a[:48, 7 * h : 7 * h + 6] = Wv_f[:, 6 * h : 6 * h + 6]
        wv_a[48, 7 * h : 7 * h + 6] = bv_f[6 * h : 6 * h + 6]
        wv_a[48, 7 * h + 6] = 1.0
    # esel[32j+6, 32j:32j+32] = 1: expands each denom row across its whole
    # 32-block (gap rows included, keeping reciprocal finite)
    esel_a = np.zeros((128, 128), np.float32)
    for j in range(4):
        esel_a[32 * j + 6, 32 * j : 32 * j + 32] = 1.0
    wop = np.zeros((2, 128, 48), f32)
    for h in range(8):
        r, j = divmod(h, 4)
        wop[r, 32 * j : 32 * j + 6, :] = np.asarray(Wo, f32)[6 * h : 6 * h + 6, :]
    # LN1's bias is skipped on-device (h1_nob = (y-mu)*rstd*g); fold it into
    # the FFN biases: b1' = b1 + ln1_b@W1 (carried as row 48 of w1 against
    # h1t's ones row), b2' = b2 + ln1_b.
    W1_f = np.asarray(W1, f32)
    b1_f = np.asarray(b1, f32) + np.asarray(ln1_b, f32) @ W1_f
    w1_a = np.concatenate([W1_f, b1_f.reshape(1, FF)], 0)
    b2c_a = (np.asarray(b2, f32) + np.asarray(ln1_b, f32)).reshape(48, 1)
    ln_a = np.stack([ln1_g, ln1_b, ln2_g, ln2_b]).astype(f32)

    weights = dict(
        wqa=wqa_a, wqb=wqb_a, wka=wka_a, wkb=wkb_a, wv=wv_a,
        esel=esel_a.astype(bf),
        wopa=wop[0].astype(bf), wopb=wop[1].astype(bf),
        boc=np.asarray(bo, f32).reshape(48, 1),
        w1=w1_a, w2=np.ascontiguousarray(np.asarray(W2, f32)),
        b2c=b2c_a,
        lnc=np.ascontiguousarray(ln_a),
    )
    in_maps = []
    for c in range(ncores):
        m = dict(weights)
        sh = xta[c * wpc : (c + 1) * wpc]  # [wpc, 50, T]
        m["xta"] = np.ascontiguousarray(
            sh.transpose(1, 0, 2).reshape(50, wpc * T)
        )
        in_maps.append(m)
    return in_maps, win, slot


def kernel(**inputs):
    key = ("full", WPC)
    if key not in _CACHE:
        _CACHE[key] = _build_bass(WPC)
    nc = _CACHE[key]
    in_maps, win, slot = _prep_host(**inputs)
    from concourse import bass_utils

    r = bass_utils.run_bass_kernel_spmd(
        nc, in_maps, core_ids=list(range(NCORES))
    )
    full = np.concatenate([r.results[c]["out"] for c in range(NCORES)], 0)
    return full[win, slot].astype(np.float32)

